# revision 1
# baseline (speedup 1.0000x reference)
"""Distributed single-head attention kernel for one TRN2 chip (8 NeuronCores).

Problem: x[8192,1024] fp32; q/k/v = x@W* + b*; out = softmax(q k^T / 8) @ v.

Strategy (sequence parallel, fully collective-free):
  - shard the QUERY rows of x across 8 cores; REPLICATE x (bf16, 16MB)
    to every core so each computes all of k and v locally. Measured on
    this part, the first collective cannot start before ~65-75us (CC
    runtime rendezvous gated by cross-core start skew, independent of
    trigger time), which idled the PE for ~30us; recomputing k/v for
    the 7 remote shards costs ~35us of PE work that replaces that idle
    and removes all CC latency/variance and the end-of-kernel CC sync
  - HOST pre-packs the inputs: x cast to bf16 and pre-transposed into
    the exact [partition, shard, chunk, m] SBUF layout (16KB DMA lines),
    ROTATED per core so shard slot 0 is always the core's own rows (no
    device-side rank logic); weights pre-cast to bf16 with Wk|Wv packed
    into one [128, 128] lhsT so k and v project in a single matmul chain
  - shard tiles stream in round-robin over the three DMA queues
    (sync/scalar/gpsimd) and recycle through a 4-deep pool
  - attention is computed transposed: S^T[n,m] = K @ q^T so softmax's
    n-dimension lands on partitions; the row-sum comes free from a ones
    column appended to V (V_aug): out^T = V_aug^T @ E^T accumulates
    numerator and denominator in one PSUM chain
  - the packed kvT tile is used directly as the S-matmul lhsT with the
    contraction padded to K=128 (qT's bottom 64 partitions are zeroed,
    nulling the v-junk rows): the HAM clock gate does not count K=64
    matmuls as PE-busy and would hold the PE at 1.2 GHz for the whole
    attention loop; K=128 padding keeps it at 2.4 GHz at zero cycle cost
  - exp alternates between ScalarE (native) and VectorE (Schraudolph
    bit-trick emitting the bf16 pattern via an int16 convert)
  - finalize: transpose out^T back (bf16), normalize by reciprocal
    row-sum, +bv

Math shortcuts (exactness preserved):
  - softmax(s + c_row) == softmax(s): the k-bias term is row-constant -> bk
    dropped entirely
  - softmax rows sum to 1 -> v-bias added after the weighted sum
  - logits are ~N(0,1), exp cannot overflow in fp32 -> no max pass
  - k/v recomputed locally are bitwise identical to what a gather would
    deliver (same bf16 inputs, same matmul), so accuracy is unchanged
"""

import sys

if "/opt/trn_rl_repo" not in sys.path:
    sys.path.insert(0, "/opt/trn_rl_repo")

import math

import numpy as np

N, D, H = 8192, 1024, 64
NCORES = 8
ML = N // NCORES          # rows per core: 1024
P = 128
CCH = D // P              # contraction chunks over D: 8
MT = ML // P              # 128-row tiles per shard: 8
NCH = N // P              # total key chunks of 128: 64
SCALE = float(H) ** -0.5
PIPE_D = 4                # V-matmul runs this many chunks behind the S/exp
SH_COLS = CCH * ML        # flattened xT columns per shard (8192)

# Schraudolph exp producing a bf16 bit pattern in int16:
#   bf16_bits(exp(scale*s)) ~= round(A16*s + B16)
A16 = SCALE * math.log2(math.e) * 2.0**7
B16 = 127.0 * 2.0**7 - 0.06 * 2.0**7   # c=0.06 tuned for end-to-end error

_CACHE = {}


def _build():
    from concourse import bacc, mybir, tile, masks

    F32 = mybir.dt.float32
    BF16 = mybir.dt.bfloat16
    I16 = mybir.dt.int16
    AF = mybir.ActivationFunctionType
    ADD = mybir.AluOpType.add
    MULT = mybir.AluOpType.mult

    nc = bacc.Bacc("TRN2", target_bir_lowering=False, debug=False,
                   num_devices=NCORES)

    xt_d = nc.dram_tensor("xt", [P, NCORES * SH_COLS], BF16,
                          kind="ExternalInput")
    wkv_d = nc.dram_tensor("wkv", [P, CCH * P], BF16, kind="ExternalInput")
    wq_d = nc.dram_tensor("wq", [P, CCH * H], BF16, kind="ExternalInput")
    bq_d = nc.dram_tensor("bq", [H, 1], F32, kind="ExternalInput")
    bv_d = nc.dram_tensor("bv", [1, H], F32, kind="ExternalInput")
    out_d = nc.dram_tensor("out", [ML, H], F32, kind="ExternalOutput")

    with tile.TileContext(nc) as tc:
        with (
            tc.tile_pool(name="constp", bufs=1) as constp,
            tc.tile_pool(name="wtsp", bufs=1) as wtsp,
            tc.tile_pool(name="xinp", bufs=4) as xinp,
            tc.tile_pool(name="qkvp", bufs=1) as qkvp,
            tc.tile_pool(name="eTp", bufs=16) as eTp,
            tc.tile_pool(name="finp", bufs=2) as finp,
        ):
            # ---- weight + bias loads (small, gpsimd queue first) ----
            wkv_sb = wtsp.tile([P, CCH * P], BF16, tag="wkv")
            nc.gpsimd.dma_start(wkv_sb[:], wkv_d[:, :])
            wq_sb = wtsp.tile([P, CCH * H], BF16, tag="wq")
            nc.gpsimd.dma_start(wq_sb[:], wq_d[:, :])
            bq_sb = constp.tile([H, 1], F32, tag="bq")
            nc.gpsimd.dma_start(bq_sb[:], bq_d[:, :])
            bv_sb = constp.tile([1, H], F32, tag="bv")
            nc.gpsimd.dma_start(bv_sb[:], bv_d[:, :])

            # ---- x shard loads: round-robin over the 3 DMA queues.
            # Shard 0 (own rows, needed first for q + first chunks) is
            # split across sync+scalar so it lands in ~8us.
            xh = []
            for j in range(NCORES):
                xj = xinp.tile([P, SH_COLS], BF16, tag="xh", name=f"xh_{j}")
                src = xt_d[:, SH_COLS * j:SH_COLS * (j + 1)]
                if j == 0:
                    nc.sync.dma_start(xj[0:64, :], src[0:64, :])
                    nc.scalar.dma_start(xj[64:P, :], src[64:P, :])
                else:
                    eng = (nc.gpsimd, nc.sync, nc.scalar)[j % 3]
                    eng.dma_start(xj[:], src)
                xh.append(xj)

            # ---- constants ----
            id_bf = constp.tile([P, P], BF16, tag="id_bf")
            masks.make_identity(nc, id_bf[:])
            ones1 = constp.tile([1, P], F32, tag="ones1")
            nc.vector.memset(ones1[:], 1.0)
            bvb = constp.tile([P, H], F32, tag="bvb")  # bv broadcast to rows

            # packed kvT for all shards: rows 0:64 = kT, 64:128 = vT;
            # used directly as the (K=128-padded) S-matmul lhsT
            kvT_all = qkvp.tile([P, NCORES, ML], BF16, tag="kvT")
            # v natural [key, h|1] for all shards (ones col for row-sums)
            v_all = qkvp.tile([P, NCH, H + 1], BF16, tag="v_nat")
            nc.vector.memset(v_all[:, :, H:H + 1], 1.0)
            # qT padded to 128 partitions with a zero bottom half (see
            # module docstring: K=128 keeps the HAM clock gate warm)
            qT_sb = qkvp.tile([P, ML], BF16, tag="qT")
            nc.vector.memset(qT_sb[H:P, :], 0.0)

            # PSUM budget is exactly 8 banks: sT 2 bufs x 2 banks, oT 2
            # banks, and one shared 2-buf pool (1 bank each) that the
            # kv/q accumulator chains, v transposes and bvb rotate through
            with (
                tc.tile_pool(name="ps_a", bufs=2, space="PSUM") as ps_a,
                tc.tile_pool(name="ps_sT", bufs=2, space="PSUM") as ps_sT,
                tc.tile_pool(name="ps_oT", bufs=1, space="PSUM") as ps_oT,
            ):
                def kv_proj(j):
                    # packed [Wk|Wv] lhsT: k and v in one matmul chain.
                    # chunk-outer order runs both m-half matmuls under one
                    # weight load (halves the LDWEIGHTS count)
                    accs = [ps_a.tile([P, 512], F32, tag="acc",
                                      name=f"acc_kv_{j}_{h2}")
                            for h2 in range(2)]
                    for ch in range(CCH):
                        for h2 in range(2):
                            nc.tensor.matmul(
                                accs[h2][:], wkv_sb[:, P * ch:P * (ch + 1)],
                                xh[j][:, ML * ch + 512 * h2:
                                      ML * ch + 512 * (h2 + 1)],
                                start=(ch == 0), stop=(ch == CCH - 1))
                    for h2 in range(2):
                        eng = nc.scalar if (j + h2) % 2 == 0 else nc.vector
                        (eng.copy if eng is nc.scalar
                         else eng.tensor_copy)(
                            kvT_all[:, j, 512 * h2:512 * (h2 + 1)],
                            accs[h2][:])
                    # v natural tiles via PE transpose (identity block at
                    # partitions 64:128 matches the v rows' base partition)
                    for t in range(MT):
                        vps = ps_a.tile([P, H], BF16, tag="acc",
                                        name=f"vps_{j}_{t}")
                        nc.tensor.transpose(
                            vps[:], kvT_all[H:P, j, P * t:P * (t + 1)],
                            id_bf[H:P, H:P])
                        eng = nc.scalar if t % 2 == 0 else nc.vector
                        (eng.copy if eng is nc.scalar
                         else eng.tensor_copy)(
                            v_all[:, MT * j + t, 0:H], vps[:])

                # q projection from shard 0 (the core's own rows)
                kv_proj(0)
                qaccs = [ps_a.tile([H, 512], F32, tag="acc",
                                   name=f"acc_q_{h2}") for h2 in range(2)]
                for ch in range(CCH):
                    for h2 in range(2):
                        nc.tensor.matmul(
                            qaccs[h2][:], wq_sb[:, H * ch:H * (ch + 1)],
                            xh[0][:, ML * ch + 512 * h2:
                                  ML * ch + 512 * (h2 + 1)],
                            start=(ch == 0), stop=(ch == CCH - 1))
                for h2 in range(2):
                    nc.vector.tensor_scalar_add(qT_sb[0:H, 512 * h2:
                                                      512 * (h2 + 1)],
                                                qaccs[h2][:], bq_sb[:])

                # bv broadcast via rank-1 matmul: ones[1,128]^T @ bv[1,64]
                bvb_ps = ps_a.tile([P, H], F32, tag="acc")
                nc.tensor.matmul(bvb_ps[:], ones1[:], bv_sb[:],
                                 start=True, stop=True)
                nc.vector.tensor_copy(bvb[:], bvb_ps[:])

                # ---- attention: S^T = K qT; E^T = exp(S^T/8);
                #      O^T += Vaug^T E^T, pipelined PIPE_D chunks behind
                oT = ps_oT.tile([H + 1, ML], F32, tag="oT")
                eTs = []

                def chunk(i):
                    j, c = divmod(i, MT)
                    sT = ps_sT.tile([P, ML], F32, tag="sT", name=f"sT_{i}")
                    for h2 in range(2):
                        msl = slice(512 * h2, 512 * (h2 + 1))
                        nc.tensor.matmul(
                            sT[:, msl], kvT_all[:, j, P * c:P * (c + 1)],
                            qT_sb[:, msl], start=True, stop=True)
                    if i >= NCH - 2:
                        # last chunks are on the end-of-kernel critical
                        # path: split their exp across both engines
                        eTi = eTp.tile([P, ML], I16, tag="eT", name=f"eTi_{i}")
                        eTb = eTi.bitcast(BF16)
                        nc.scalar.activation(eTb[:, 0:512], sT[:, 0:512],
                                             AF.Exp, scale=SCALE)
                        nc.vector.tensor_scalar(eTi[:, 512:ML],
                                                sT[:, 512:ML], A16, B16,
                                                op0=MULT, op1=ADD)
                        eTs.append(eTb)
                    elif i % 2 == 0:
                        eT = eTp.tile([P, ML], BF16, tag="eT", name=f"eT_{i}")
                        nc.scalar.activation(eT[:], sT[:], AF.Exp, scale=SCALE)
                        eTs.append(eT)
                    else:
                        eTi = eTp.tile([P, ML], I16, tag="eT", name=f"eTi_{i}")
                        nc.vector.tensor_scalar(eTi[:], sT[:], A16, B16,
                                                op0=MULT, op1=ADD)
                        eTs.append(eTi.bitcast(BF16))
                    if i >= PIPE_D:
                        _accum_v(nc, oT, v_all, eTs[i - PIPE_D], i - PIPE_D)

                # per shard: kv projection, then its 8 chunks — keeps the
                # PE stream dense while later shards' DMAs are in flight
                for i in range(MT):
                    chunk(i)
                for j in range(1, NCORES):
                    kv_proj(j)
                    for i in range(MT * j, MT * (j + 1)):
                        chunk(i)
                for i in range(NCH - PIPE_D, NCH):
                    _accum_v(nc, oT, v_all, eTs[i], i)

                # ---- finalize: transpose back (bf16), normalize, +bv ----
                oT_sb = qkvp.tile([H + 1, ML], BF16, tag="oT_sb")
                for t in range(MT):
                    eng = nc.scalar if t % 2 == 0 else nc.vector
                    (eng.copy if eng is nc.scalar else eng.tensor_copy)(
                        oT_sb[:, P * t:P * (t + 1)], oT[:, P * t:P * (t + 1)])
                for t in range(MT):
                    ft = ps_sT.tile([P, H + 1], BF16, tag="sT",
                                    name=f"ft_{t}")
                    nc.tensor.transpose(
                        ft[:], oT_sb[:, P * t:P * (t + 1)],
                        id_bf[:H + 1, :H + 1])
                    rcp = finp.tile([P, 1], F32, tag="rcp", name=f"rcp_{t}")
                    nc.vector.reciprocal(rcp[:], ft[:, H:H + 1])
                    res = finp.tile([P, H], F32, tag="res", name=f"res_{t}")
                    # fused (numerator * 1/rowsum) + bv in one DVE op
                    nc.vector.scalar_tensor_tensor(
                        res[:], ft[:, 0:H], rcp[:], bvb[:],
                        op0=MULT, op1=ADD)
                    eng = nc.sync if t % 2 == 0 else nc.scalar
                    eng.dma_start(out_d[P * t:P * (t + 1), :], res[:])

    nc.compile()
    return nc


def _accum_v(nc, oT, v_all, eT, i):
    for h2 in range(2):
        msl = slice(512 * h2, 512 * (h2 + 1))
        nc.tensor.matmul(oT[:, msl], v_all[:, i, :], eT[:, msl],
                         start=(i == 0), stop=(i == NCH - 1),
                         skip_group_check=True)


def _get_nc():
    if "nc" not in _CACHE:
        _CACHE["nc"] = _build()
    return _CACHE["nc"]


def _prep_inputs(inputs):
    import ml_dtypes

    bf16 = ml_dtypes.bfloat16
    wkv = np.concatenate(
        [np.asarray(inputs["Wk"], dtype=np.float32),
         np.asarray(inputs["Wv"], dtype=np.float32)], axis=1).astype(bf16)
    wkv_p = np.ascontiguousarray(
        wkv.reshape(CCH, P, P).transpose(1, 0, 2).reshape(P, CCH * P))
    wq = np.asarray(inputs["Wq"], dtype=np.float32).astype(bf16)
    wq_p = np.ascontiguousarray(
        wq.reshape(CCH, P, H).transpose(1, 0, 2).reshape(P, CCH * H))
    bq = np.ascontiguousarray(
        inputs["bq"], dtype=np.float32).reshape(H, 1)
    bv = np.ascontiguousarray(
        inputs["bv"], dtype=np.float32).reshape(1, H)

    x = np.asarray(inputs["x"], dtype=np.float32)
    # per-shard packed layout [p, c*1024+m] with 16KB-contiguous lines
    shards = []
    for j in range(NCORES):
        xs = x[ML * j:ML * (j + 1)].astype(bf16)      # [m, d]
        t = xs.T.reshape(CCH, P, ML)                  # [c, p, m]
        shards.append(np.ascontiguousarray(
            t.transpose(1, 0, 2).reshape(P, SH_COLS)))
    in_maps = []
    for i in range(NCORES):
        # rotate so shard slot 0 is core i's own rows — the kernel then
        # needs no device-side rank logic; key order differs per core but
        # softmax sums over all keys, so the result is unchanged
        xt = np.concatenate([shards[(i + j) % NCORES]
                             for j in range(NCORES)], axis=1)
        in_maps.append({
            "xt": np.ascontiguousarray(xt), "wkv": wkv_p, "wq": wq_p,
            "bq": bq, "bv": bv,
        })
    return in_maps


def _run(inputs, trace=False, **kw):
    from concourse.bass_utils import run_bass_kernel_spmd

    nc = _get_nc()
    in_maps = _prep_inputs(inputs)
    res = run_bass_kernel_spmd(nc, in_maps, core_ids=list(range(NCORES)),
                               trace=trace, **kw)
    out = np.concatenate([res.results[i]["out"] for i in range(NCORES)],
                         axis=0)
    return out, res


def kernel(x, Wq, bq, Wk, bk, Wv, bv):
    out, _ = _run({"x": x, "Wq": Wq, "bq": bq, "Wk": Wk, "Wv": Wv, "bv": bv})
    return out



# revision 22
# speedup vs baseline: 1.2279x; 1.2279x over previous
"""Distributed single-head attention kernel for one TRN2 chip (8 NeuronCores).

Problem: x[8192,1024] fp32; q/k/v = x@W* + b*; out = softmax(q k^T / 8) @ v.

Strategy (sequence parallel, fully collective-free), v2:
  - shard the QUERY rows of x across 8 cores; REPLICATE x (bf16, 16MB)
    to every core so each computes all of k and v locally (measured here:
    the first collective cannot start before ~65-75us, so recomputing
    k/v locally beats any all-gather).
  - HOST pre-packs: x cast to bf16, pre-transposed to the [partition,
    shard, chunk, m] SBUF layout, ROTATED per core so shard slot 0 is the
    core's own rows; within each shard the 128-row KEY blocks are
    interleaved [0,2,4,6,1,3,5,7] so even blocks sit in m-half 0 and odd
    blocks in m-half 1 (softmax is key-order invariant; the host undoes
    the query-side permutation when unsharding).
  - kv projection per shard with TWO packed weight blocks:
    wkvA=[Wk|Wv] for m-half 0 (k on partitions 0:64, v on 64:128) and
    wkvB=[Wv|Wk] for m-half 1 (v top, k bottom).  This puts the k-rows of
    odd key blocks on partitions 64:128, which lets the S matmul either
    (a) run K=128-padded against a zero-padded q (HAM-warm), or
    (b) ROWTILE: run even/odd key blocks CONCURRENTLY in the top/bottom
    halves of the PE array (tile_position row groups, K=64 each),
    halving S-matmul time.
  - attention computed transposed: S^T[n,m] = K q^T; row-sums come free
    from a ones column in V_aug: O^T = V_aug^T E^T in one PSUM chain.
  - exp alternates ScalarE (native exp) and VectorE (Schraudolph bit
    trick); optionally E and v are stored fp8e4m3 and the V-accumulation
    runs in fp8 DoubleRow mode (contraction 256 = 2 key blocks fused).
  - x streams in chunk-pair granular DMAs across the 3 DGE queues
    (sync/scalar/gpsimd) ordered so shard 0 + weights land first; the
    kv/q chains consume chunk-by-chunk so the PE starts ~10us earlier.
  - finalize WITHOUT transposes: out stays transposed [H, ML] in DRAM
    (host transposes when unsharding).  bias enters as a rank-1 matmul
    bv (x) rowsum into the numerator PSUM; the denominator broadcast is a
    ones (x) den matmul; one reciprocal + one multiply finish it.

Math shortcuts (exactness preserved):
  - softmax(s + c_row) == softmax(s): bk dropped entirely
  - softmax rows sum to 1 -> v-bias applied as (num + bv (x) den)/den
  - logits ~N(0,1): exp cannot overflow fp32/bf16 -> no max pass
"""

import sys

if "/opt/trn_rl_repo" not in sys.path:
    sys.path.insert(0, "/opt/trn_rl_repo")

import math

import numpy as np

N, D, H = 8192, 1024, 64
NCORES = 8
ML = N // NCORES          # rows per core: 1024
P = 128
CCH = D // P              # contraction chunks over D: 8
MT = ML // P              # 128-row key blocks per shard: 8
NPAIR = MT // 2           # key-block pairs per shard: 4
NCH = N // P              # total key blocks: 64
SCALE = float(H) ** -0.5
PIPE_P = 2                # V-matmul runs this many PAIRS behind S/exp

# variant flags (module level so tests can flip them before _get_nc())
ROWTILE = True            # S matmul as two concurrent K=64 row-tiles
FP8V = False              # E & v in fp8e4m3, V-accum in DoubleRow mode
FP8_U8_TS = True          # vector exp path: direct uint8 tensor_scalar
                          # (requires saturating fp32->uint8 convert)

# Schraudolph exp bit patterns.
#   bf16: bits16 = A16*sraw + B16  (c tuned for end-to-end error)
A16 = SCALE * math.log2(math.e) * 2.0**7
B16 = 127.0 * 2.0**7 - 0.06 * 2.0**7
#   fp8e4m3: bits8 = A8*sraw + B8
A8 = SCALE * math.log2(math.e) * 2.0**3
B8 = 7.0 * 2.0**3 - 0.06 * 2.0**3

_CACHE = {}


def _build(rowtile, fp8v, fp8_u8):
    from concourse import bacc, mybir, tile, masks

    F32 = mybir.dt.float32
    BF16 = mybir.dt.bfloat16
    I16 = mybir.dt.int16
    U8 = mybir.dt.uint8
    F8 = mybir.dt.float8e4
    AF = mybir.ActivationFunctionType
    ADD = mybir.AluOpType.add
    MULT = mybir.AluOpType.mult
    DR = mybir.MatmulPerfMode.DoubleRow

    nc = bacc.Bacc("TRN2", target_bir_lowering=False, debug=False,
                   num_devices=NCORES)

    SH_COLS = CCH * ML        # flattened xT columns per shard (8192)

    xt_d = nc.dram_tensor("xt", [P, NCORES * SH_COLS], BF16,
                          kind="ExternalInput")
    # wkv[:, 0] = [Wk|Wv] chunk-major, wkv[:, 1] = [Wv|Wk]
    wkv_d = nc.dram_tensor("wkv", [P, 2, CCH * P], BF16,
                           kind="ExternalInput")
    wq_d = nc.dram_tensor("wq", [P, CCH * H], BF16, kind="ExternalInput")
    bq_d = nc.dram_tensor("bq", [1, H], BF16, kind="ExternalInput")
    bv_d = nc.dram_tensor("bv", [1, H], BF16, kind="ExternalInput")
    out_d = nc.dram_tensor("out", [H, ML], F32, kind="ExternalOutput")

    with tile.TileContext(nc) as tc:
        with (
            tc.tile_pool(name="constp", bufs=1) as constp,
            tc.tile_pool(name="wtsp", bufs=1) as wtsp,
            tc.tile_pool(name="xinp", bufs=4) as xinp,
            tc.tile_pool(name="qkvp", bufs=1) as qkvp,
            tc.tile_pool(name="eTp", bufs=8) as eTp,
            tc.tile_pool(name="vstp", bufs=2) as vstp,
            tc.tile_pool(name="finp", bufs=1) as finp,
        ):
            # ---- weight + bias + x loads --------------------------------
            # 3 DGE queues: sync, scalar, gpsimd.  Weights first (small),
            # then shard 0's chunk-pairs striped across all three so the
            # kv/q chains can start ~10us earlier; bulk shards follow.
            # Scalar gets only early, wait-free triggers (its queue later
            # runs the exp stream and a blocked DMA wait would stall it).
            wkv_sb = wtsp.tile([P, 2, CCH * P], BF16, tag="wkv")
            nc.sync.dma_start(wkv_sb[:], wkv_d[:, :, :])
            wq_sb = wtsp.tile([P, CCH * H], BF16, tag="wq")
            nc.scalar.dma_start(wq_sb[:], wq_d[:, :])
            # bq/bv ship as [1, H] (single-descriptor DMAs; a [H, 1]
            # layout costs 64 four-byte descriptors and stalls the queue)
            bq_sb = constp.tile([1, H], BF16, tag="bq")
            nc.scalar.dma_start(bq_sb[:], bq_d[:, :])
            bv_sb = constp.tile([1, H], BF16, tag="bv")
            nc.scalar.dma_start(bv_sb[:], bv_d[:, :])

            xh = [xinp.tile([P, SH_COLS], BF16, tag="xh", name=f"xh_{j}")
                  for j in range(NCORES)]

            def xsrc(j, lo, hi):
                return xt_d[:, SH_COLS * j + lo:SH_COLS * j + hi]

            HALF = SH_COLS // 2
            QTR = SH_COLS // 4
            # shard 0 split sync/gpsimd (the scalar DMA queue moves bulk
            # data ~3-4x slower than sync/gpsimd — measured — so it only
            # ever carries the small weights and SBUF-local copies)
            nc.gpsimd.dma_start(xh[0][:, 0:QTR], xsrc(0, 0, QTR))
            nc.sync.dma_start(xh[0][:, QTR:HALF], xsrc(0, QTR, HALF))
            nc.gpsimd.dma_start(xh[0][:, HALF:3 * QTR],
                                xsrc(0, HALF, 3 * QTR))
            nc.sync.dma_start(xh[0][:, 3 * QTR:SH_COLS],
                              xsrc(0, 3 * QTR, SH_COLS))

            def x_load(j):
                # halves on sync+gpsimd, shard-major order
                nc.sync.dma_start(xh[j][:, 0:HALF], xsrc(j, 0, HALF))
                nc.gpsimd.dma_start(xh[j][:, HALF:SH_COLS],
                                    xsrc(j, HALF, SH_COLS))

            for j in (1, 2, 3):
                x_load(j)
            # shards 4-7 recycle bufs 0-3; their triggers wait on buffer
            # release, so they are EMITTED LAZILY (in the shard loop
            # below) to avoid parking a blocked trigger at a queue head.

            # ---- constants ----------------------------------------------
            id_bf = constp.tile([P, P], BF16, tag="id_bf")
            masks.make_identity(nc, id_bf[:])
            ones1 = constp.tile([1, 512], BF16, tag="ones1")
            nc.vector.memset(ones1[:], 1.0)

            # kvT for all shards: [p, shard, key-block, col]
            #   even blocks (m-half 0): k on partitions 0:64, v on 64:128
            #   odd  blocks (m-half 1): v on partitions 0:64, k on 64:128
            kvT_all = qkvp.tile([P, NCORES, MT, P], BF16, tag="kvT")
            # v natural [key, h|1] tiles; ones col feeds the row-sums
            if fp8v:
                v_all = qkvp.tile([P, NCORES * NPAIR, 2, H + 1], F8,
                                  tag="v_nat")
                nc.vector.memset(v_all[:, :, :, H:H + 1], 1.0)
            else:
                v_all = qkvp.tile([P, NCH, H + 1], BF16, tag="v_nat")
                nc.vector.memset(v_all[:, :, H:H + 1], 1.0)
            # qTa = [q; 0] (for even key blocks), qTb = [0; q] (odd)
            qTa = qkvp.tile([P, ML], BF16, tag="qTa")
            qTb = qkvp.tile([P, ML], BF16, tag="qTb")
            if not rowtile:
                nc.vector.memset(qTa[H:P, :], 0.0)
                nc.vector.memset(qTb[0:H, :], 0.0)

            with (
                tc.tile_pool(name="ps_a", bufs=2, space="PSUM") as ps_a,
                tc.tile_pool(name="ps_sT", bufs=2, space="PSUM") as ps_sT,
                tc.tile_pool(name="ps_oT", bufs=1, space="PSUM") as ps_oT,
            ):
                # ---- shard 0: kv + q interleaved chunk-by-chunk --------
                kv_accs = [ps_a.tile([P, 512], F32, tag="acc",
                                     name=f"acc_kv0_{h2}")
                           for h2 in range(2)]
                q_accs = [ps_sT.tile([H, 512], F32, tag="sT",
                                     name=f"acc_q_{h2}")
                          for h2 in range(2)]
                for ch in range(CCH):
                    for h2 in range(2):
                        nc.tensor.matmul(
                            kv_accs[h2][:],
                            wkv_sb[:, h2, P * ch:P * (ch + 1)],
                            xh[0][:, ML * ch + 512 * h2:
                                  ML * ch + 512 * (h2 + 1)],
                            start=(ch == 0), stop=(ch == CCH - 1))
                    for h2 in range(2):
                        nc.tensor.matmul(
                            q_accs[h2][:], wq_sb[:, H * ch:H * (ch + 1)],
                            xh[0][:, ML * ch + 512 * h2:
                                  ML * ch + 512 * (h2 + 1)],
                            start=(ch == 0), stop=(ch == CCH - 1))

                def kv_finish(j, kv_accs):
                    # PSUM -> SBUF kvT copies, then v natural tiles
                    for h2 in range(2):
                        eng = nc.scalar if (j + h2) % 2 == 0 else nc.vector
                        (eng.copy if eng is nc.scalar
                         else eng.tensor_copy)(
                            kvT_all[:, j, 4 * h2:4 * (h2 + 1), :],
                            kv_accs[h2][:])
                    # v natural via PE transposes (XBAR SBUF->SBUF DMA
                    # tried instead: its 256B-descriptor transfers choke
                    # the shared DMA engines and starve the x stream)
                    for b in range(MT):
                        # even blocks: v at 64:128; odd: v at 0:64
                        rows = slice(H, P) if b < 4 else slice(0, H)
                        vps = ps_a.tile([P, H], BF16, tag="acc",
                                        name=f"vps_{j}_{b}")
                        nc.tensor.transpose(
                            vps[:], kvT_all[rows, j, b, :],
                            id_bf[rows, rows])
                        eng = nc.scalar if b % 2 == 0 else nc.vector
                        if fp8v:
                            pr, i = (b, 0) if b < 4 else (b - 4, 1)
                            dst = v_all[:, NPAIR * j + pr, i, 0:H]
                        else:
                            dst = v_all[:, MT * j + b, 0:H]
                        (eng.copy if eng is nc.scalar
                         else eng.tensor_copy)(dst, vps[:])

                kv_finish(0, kv_accs)
                # bq enters as a rank-1 accumulation bq (x) ones into the
                # q PSUM chain (avoids a [H,1]-layout bias DMA)
                for h2 in range(2):
                    nc.tensor.matmul(q_accs[h2][:], bq_sb[:], ones1[:],
                                     start=False, stop=True,
                                     skip_group_check=True)
                for h2 in range(2):
                    nc.vector.tensor_copy(
                        qTa[0:H, 512 * h2:512 * (h2 + 1)], q_accs[h2][:])
                # duplicate q rows into qTb's bottom half (SBUF->SBUF
                # DMA on scalar: its queue is empty by now, so the dup
                # lands right after qTa is written)
                nc.scalar.dma_start(qTb[H:P, :], qTa[0:H, :])

                # ---- attention ------------------------------------------
                oT = ps_oT.tile([H + 1, ML], F32, tag="oT")
                epairs = []   # (eT tile-ish, global pair idx)

                def accum_v(ep, gp):
                    if fp8v:
                        eTb, = ep
                        for h2 in range(2):
                            msl = slice(512 * h2, 512 * (h2 + 1))
                            nc.tensor.matmul(
                                oT[:, msl], v_all[:, gp, :, :],
                                eTb[:, :, msl],
                                start=(gp == 0),
                                stop=(gp == NCORES * NPAIR - 1),
                                perf_mode=DR, skip_group_check=True)
                    else:
                        eA, eB, gbA, gbB = ep
                        for eT, gb in ((eA, gbA), (eB, gbB)):
                            for h2 in range(2):
                                msl = slice(512 * h2, 512 * (h2 + 1))
                                nc.tensor.matmul(
                                    oT[:, msl], v_all[:, gb, :],
                                    eT[:, msl],
                                    start=(gb == 0), stop=(gb == NCH - 1),
                                    skip_group_check=True)

                def attn_pair(j, pr):
                    gp = NPAIR * j + pr
                    last = gp == NCORES * NPAIR - 1
                    sA = ps_sT.tile([P, ML], F32, tag="sT",
                                    name=f"sA_{gp}")
                    sB = ps_sT.tile([P, ML], F32, tag="sT",
                                    name=f"sB_{gp}")
                    for h2 in range(2):
                        msl = slice(512 * h2, 512 * (h2 + 1))
                        if rowtile:
                            nc.tensor.matmul(
                                sA[:, msl], kvT_all[0:H, j, pr, :],
                                qTa[0:H, msl], start=True, stop=True,
                                tile_position=(0, 0))
                            nc.tensor.matmul(
                                sB[:, msl], kvT_all[H:P, j, 4 + pr, :],
                                qTb[H:P, msl], start=True, stop=True,
                                tile_position=(64, 0))
                        else:
                            nc.tensor.matmul(
                                sA[:, msl], kvT_all[:, j, pr, :],
                                qTa[:, msl], start=True, stop=True)
                            nc.tensor.matmul(
                                sB[:, msl], kvT_all[:, j, 4 + pr, :],
                                qTb[:, msl], start=True, stop=True)
                    # exp: block A on ScalarE (native), block B on
                    # VectorE (Schraudolph).  Last pair: split each
                    # block's halves across both engines (critical path).
                    if fp8v:
                        eTi = eTp.tile([P, 2, ML], U8, tag="eT",
                                       name=f"eT_{gp}")
                        eTb = eTi.bitcast(F8)
                        if last:
                            nc.scalar.activation(eTb[:, 0, 0:512],
                                                 sA[:, 0:512], AF.Exp,
                                                 scale=SCALE)
                            nc.vector.tensor_scalar(
                                eTi[:, 0, 512:ML], sA[:, 512:ML],
                                A8, B8, op0=MULT, op1=ADD)
                            nc.scalar.activation(eTb[:, 1, 0:512],
                                                 sB[:, 0:512], AF.Exp,
                                                 scale=SCALE)
                            nc.vector.tensor_scalar(
                                eTi[:, 1, 512:ML], sB[:, 512:ML],
                                A8, B8, op0=MULT, op1=ADD)
                        else:
                            nc.scalar.activation(eTb[:, 0, :], sA[:],
                                                 AF.Exp, scale=SCALE)
                            nc.vector.tensor_scalar(
                                eTi[:, 1, :], sB[:], A8, B8,
                                op0=MULT, op1=ADD)
                        epairs.append(((eTb,), gp))
                    else:
                        eA = eTp.tile([P, ML], BF16, tag="eT",
                                      name=f"eA_{gp}")
                        eBi = eTp.tile([P, ML], I16, tag="eT",
                                       name=f"eB_{gp}")
                        eB = eBi.bitcast(BF16)
                        if last:
                            nc.scalar.activation(eA[:, 0:512],
                                                 sA[:, 0:512], AF.Exp,
                                                 scale=SCALE)
                            nc.vector.tensor_scalar(
                                eA.bitcast(I16)[:, 512:ML], sA[:, 512:ML],
                                A16, B16, op0=MULT, op1=ADD)
                            nc.scalar.activation(eB[:, 0:512],
                                                 sB[:, 0:512], AF.Exp,
                                                 scale=SCALE)
                            nc.vector.tensor_scalar(
                                eBi[:, 512:ML], sB[:, 512:ML],
                                A16, B16, op0=MULT, op1=ADD)
                        else:
                            nc.scalar.activation(eA[:], sA[:], AF.Exp,
                                                 scale=SCALE)
                            nc.vector.tensor_scalar(eBi[:], sB[:],
                                                    A16, B16,
                                                    op0=MULT, op1=ADD)
                        epairs.append(
                            ((eA, eB, MT * j + pr, MT * j + 4 + pr), gp))
                    if len(epairs) > PIPE_P:
                        accum_v(*epairs[-1 - PIPE_P])

                def kv_chunk(j, ch, kv_accs):
                    for h2 in range(2):
                        nc.tensor.matmul(
                            kv_accs[h2][:],
                            wkv_sb[:, h2, P * ch:P * (ch + 1)],
                            xh[j][:, ML * ch + 512 * h2:
                                  ML * ch + 512 * (h2 + 1)],
                            start=(ch == 0), stop=(ch == CCH - 1))

                # per shard: 4 attention pairs interleaved with the next
                # shard's kv chunks, then its copies/transposes.  The
                # recycled-buffer DMAs (shards 4-7) are emitted as the
                # buffer they reuse is released (shard j-4's xh frees
                # after its kv+q reads complete).
                for j in range(NCORES):
                    nxt = None
                    if j + 1 < NCORES:
                        nxt = [ps_a.tile([P, 512], F32, tag="acc",
                                         name=f"acc_kv{j + 1}_{h2}")
                               for h2 in range(2)]
                    for pr in range(NPAIR):
                        attn_pair(j, pr)
                        if nxt is not None:
                            kv_chunk(j + 1, 2 * pr, nxt)
                            kv_chunk(j + 1, 2 * pr + 1, nxt)
                    if nxt is not None:
                        kv_finish(j + 1, nxt)
                    # recycled-buffer x DMAs, emitted after kv_finish so
                    # the v-transpose DMAs stay ahead of them in-queue
                    if j + 4 < NCORES:
                        x_load(j + 4)
                for ep in epairs[-PIPE_P:]:
                    accum_v(*ep)

                # ---- finalize (stay transposed) -------------------------
                # den row -> SBUF (bf16), numerator += bv (x) den,
                # den broadcast via ones (x) den, reciprocal, multiply.
                den_sb = finp.tile([1, ML], BF16, tag="den")
                nc.vector.tensor_copy(den_sb[:], oT[H:H + 1, :])
                for h2 in range(2):
                    msl = slice(512 * h2, 512 * (h2 + 1))
                    nc.tensor.matmul(oT[0:H, msl], bv_sb[:],
                                     den_sb[:, msl], start=False,
                                     stop=True, skip_group_check=True)
                denb_ps = ps_sT.tile([P, ML], F32, tag="sT", name="denb")
                for h2 in range(2):
                    msl = slice(512 * h2, 512 * (h2 + 1))
                    nc.tensor.matmul(denb_ps[:, msl], ones1[:, 0:P],
                                     den_sb[:, msl], start=True,
                                     stop=True)
                rcpb = finp.tile([P, ML], F32, tag="rcpb")
                nc.vector.reciprocal_approx_fast(rcpb[:], denb_ps[:])
                out_f = finp.tile([H, ML], F32, tag="out_f")
                nc.vector.tensor_tensor(out_f[:], oT[0:H, :],
                                        rcpb[0:H, :], op=MULT)
                nc.sync.dma_start(out_d[:, 0:512], out_f[:, 0:512])
                nc.scalar.dma_start(out_d[:, 512:ML], out_f[:, 512:ML])

    nc.compile()
    return nc


def _get_nc():
    key = (ROWTILE, FP8V, FP8_U8_TS)
    if key not in _CACHE:
        _CACHE[key] = _build(*key)
    return _CACHE[key]


# query/key 128-row block interleave within each shard: [0,2,4,6,1,3,5,7]
_PERM = np.concatenate([np.arange(0, MT, 2), np.arange(1, MT, 2)])


def _prep_inputs(inputs):
    import ml_dtypes

    bf16 = ml_dtypes.bfloat16
    wk = np.asarray(inputs["Wk"], dtype=np.float32)
    wv = np.asarray(inputs["Wv"], dtype=np.float32)
    wkvA = np.concatenate([wk, wv], axis=1).astype(bf16)
    wkvB = np.concatenate([wv, wk], axis=1).astype(bf16)
    # chunk-major packing [P, CCH*P] then stack A/B
    packA = np.ascontiguousarray(
        wkvA.reshape(CCH, P, P).transpose(1, 0, 2).reshape(P, CCH * P))
    packB = np.ascontiguousarray(
        wkvB.reshape(CCH, P, P).transpose(1, 0, 2).reshape(P, CCH * P))
    wkv_p = np.ascontiguousarray(
        np.stack([packA, packB], axis=1))            # [P, 2, CCH*P]
    wq = np.asarray(inputs["Wq"], dtype=np.float32).astype(bf16)
    wq_p = np.ascontiguousarray(
        wq.reshape(CCH, P, H).transpose(1, 0, 2).reshape(P, CCH * H))
    bq = np.ascontiguousarray(
        np.asarray(inputs["bq"], dtype=np.float32).astype(bf16)
    ).reshape(1, H)
    bv = np.ascontiguousarray(
        np.asarray(inputs["bv"], dtype=np.float32).astype(bf16)
    ).reshape(1, H)

    x = np.asarray(inputs["x"], dtype=np.float32)
    SH_COLS = CCH * ML
    shards = []
    for j in range(NCORES):
        xs = x[ML * j:ML * (j + 1)].astype(bf16)      # [m, d]
        # interleave the 128-row blocks: even blocks first, then odd
        xs = xs.reshape(MT, P, D)[_PERM].reshape(ML, D)
        t = xs.T.reshape(CCH, P, ML)                  # [c, p, m]
        shards.append(np.ascontiguousarray(
            t.transpose(1, 0, 2).reshape(P, SH_COLS)))
    in_maps = []
    for i in range(NCORES):
        # rotate so shard slot 0 is core i's own rows
        xt = np.concatenate([shards[(i + j) % NCORES]
                             for j in range(NCORES)], axis=1)
        in_maps.append({
            "xt": np.ascontiguousarray(xt), "wkv": wkv_p, "wq": wq_p,
            "bq": bq, "bv": bv,
        })
    return in_maps


def _run(inputs, trace=False, **kw):
    from concourse.bass_utils import run_bass_kernel_spmd

    nc = _get_nc()
    in_maps = _prep_inputs(inputs)
    res = run_bass_kernel_spmd(nc, in_maps, core_ids=list(range(NCORES)),
                               trace=trace, **kw)
    inv = np.argsort(_PERM)
    parts = []
    for i in range(NCORES):
        o = res.results[i]["out"]                     # [H, ML]
        o = np.ascontiguousarray(o.T)                 # [ML, H] permuted
        parts.append(o.reshape(MT, P, H)[inv].reshape(ML, H))
    return np.concatenate(parts, axis=0), res


def kernel(x, Wq, bq, Wk, bk, Wv, bv):
    out, _ = _run({"x": x, "Wq": Wq, "bq": bq, "Wk": Wk, "Wv": Wv, "bv": bv})
    return out


# revision 25
# speedup vs baseline: 1.2387x; 1.0088x over previous
"""Distributed single-head attention kernel for one TRN2 chip (8 NeuronCores).

Problem: x[8192,1024] fp32; q/k/v = x@W* + b*; out = softmax(q k^T / 8) @ v.

Strategy (sequence parallel, fully collective-free), v2:
  - shard the QUERY rows of x across 8 cores; REPLICATE x (bf16, 16MB)
    to every core so each computes all of k and v locally (measured here:
    the first collective cannot start before ~65-75us, so recomputing
    k/v locally beats any all-gather).
  - HOST pre-packs: x cast to bf16, pre-transposed to the [partition,
    shard, chunk, m] SBUF layout, ROTATED per core so shard slot 0 is the
    core's own rows; within each shard the 128-row KEY blocks are
    interleaved [0,2,4,6,1,3,5,7] so even blocks sit in m-half 0 and odd
    blocks in m-half 1 (softmax is key-order invariant; the host undoes
    the query-side permutation when unsharding).
  - kv projection per shard with TWO packed weight blocks:
    wkvA=[Wk|Wv] for m-half 0 (k on partitions 0:64, v on 64:128) and
    wkvB=[Wv|Wk] for m-half 1 (v top, k bottom).  This puts the k-rows of
    odd key blocks on partitions 64:128, which lets the S matmul either
    (a) run K=128-padded against a zero-padded q (HAM-warm), or
    (b) ROWTILE: run even/odd key blocks CONCURRENTLY in the top/bottom
    halves of the PE array (tile_position row groups, K=64 each),
    halving S-matmul time.
  - attention computed transposed: S^T[n,m] = K q^T; row-sums come free
    from a ones column in V_aug: O^T = V_aug^T E^T in one PSUM chain.
  - exp alternates ScalarE (native exp) and VectorE (Schraudolph bit
    trick); optionally E and v are stored fp8e4m3 and the V-accumulation
    runs in fp8 DoubleRow mode (contraction 256 = 2 key blocks fused).
  - x streams in chunk-pair granular DMAs across the 3 DGE queues
    (sync/scalar/gpsimd) ordered so shard 0 + weights land first; the
    kv/q chains consume chunk-by-chunk so the PE starts ~10us earlier.
  - finalize WITHOUT transposes: out stays transposed [H, ML] in DRAM
    (host transposes when unsharding).  bias enters as a rank-1 matmul
    bv (x) rowsum into the numerator PSUM; the denominator broadcast is a
    ones (x) den matmul; one reciprocal + one multiply finish it.

Math shortcuts (exactness preserved):
  - softmax(s + c_row) == softmax(s): bk dropped entirely
  - softmax rows sum to 1 -> v-bias applied as (num + bv (x) den)/den
  - logits ~N(0,1): exp cannot overflow fp32/bf16 -> no max pass
"""

import sys

if "/opt/trn_rl_repo" not in sys.path:
    sys.path.insert(0, "/opt/trn_rl_repo")

import math

import numpy as np

N, D, H = 8192, 1024, 64
NCORES = 8
ML = N // NCORES          # rows per core: 1024
P = 128
CCH = D // P              # contraction chunks over D: 8
MT = ML // P              # 128-row key blocks per shard: 8
NPAIR = MT // 2           # key-block pairs per shard: 4
NCH = N // P              # total key blocks: 64
SCALE = float(H) ** -0.5
PIPE_P = 2                # V-matmul runs this many PAIRS behind S/exp

# variant flags (module level so tests can flip them before _get_nc())
ROWTILE = True            # S matmul as two concurrent K=64 row-tiles
FP8V = False              # E & v in fp8e4m3, V-accum in DoubleRow mode
FP8_U8_TS = True          # vector exp path: direct uint8 tensor_scalar
                          # (requires saturating fp32->uint8 convert)

# Schraudolph exp bit patterns.
#   bf16: bits16 = A16*sraw + B16  (c tuned for end-to-end error)
A16 = SCALE * math.log2(math.e) * 2.0**7
B16 = 127.0 * 2.0**7 - 0.06 * 2.0**7
#   fp8e4m3: bits8 = A8*sraw + B8
A8 = SCALE * math.log2(math.e) * 2.0**3
B8 = 7.0 * 2.0**3 - 0.06 * 2.0**3

_CACHE = {}


def _build(rowtile, fp8v, fp8_u8):
    from concourse import bacc, mybir, tile, masks

    F32 = mybir.dt.float32
    BF16 = mybir.dt.bfloat16
    I16 = mybir.dt.int16
    U8 = mybir.dt.uint8
    F8 = mybir.dt.float8e4
    AF = mybir.ActivationFunctionType
    ADD = mybir.AluOpType.add
    MULT = mybir.AluOpType.mult
    DR = mybir.MatmulPerfMode.DoubleRow

    nc = bacc.Bacc("TRN2", target_bir_lowering=False, debug=False,
                   num_devices=NCORES)

    SH_COLS = CCH * ML        # flattened xT columns per shard (8192)

    xt_d = nc.dram_tensor("xt", [P, NCORES * SH_COLS], BF16,
                          kind="ExternalInput")
    # wkv[:, 0] = [Wk|Wv] chunk-major, wkv[:, 1] = [Wv|Wk]
    wkv_d = nc.dram_tensor("wkv", [P, 2, CCH * P], BF16,
                           kind="ExternalInput")
    wq_d = nc.dram_tensor("wq", [P, CCH * H], BF16, kind="ExternalInput")
    bq_d = nc.dram_tensor("bq", [1, H], BF16, kind="ExternalInput")
    bv_d = nc.dram_tensor("bv", [1, H], BF16, kind="ExternalInput")
    out_d = nc.dram_tensor("out", [H, ML], F32, kind="ExternalOutput")

    with tile.TileContext(nc) as tc:
        with (
            tc.tile_pool(name="constp", bufs=1) as constp,
            tc.tile_pool(name="wtsp", bufs=1) as wtsp,
            tc.tile_pool(name="xinp", bufs=4) as xinp,
            tc.tile_pool(name="qkvp", bufs=1) as qkvp,
            tc.tile_pool(name="eTp", bufs=8) as eTp,
            tc.tile_pool(name="vstp", bufs=2) as vstp,
            tc.tile_pool(name="finp", bufs=1) as finp,
        ):
            # ---- weight + bias + x loads --------------------------------
            # 3 DGE queues: sync, scalar, gpsimd.  Weights first (small),
            # then shard 0's chunk-pairs striped across all three so the
            # kv/q chains can start ~10us earlier; bulk shards follow.
            # Scalar gets only early, wait-free triggers (its queue later
            # runs the exp stream and a blocked DMA wait would stall it).
            wkv_sb = wtsp.tile([P, 2, CCH * P], BF16, tag="wkv")
            nc.sync.dma_start(wkv_sb[:], wkv_d[:, :, :])
            wq_sb = wtsp.tile([P, CCH * H], BF16, tag="wq")
            nc.scalar.dma_start(wq_sb[:], wq_d[:, :])
            # bq/bv ship as [1, H] (single-descriptor DMAs; a [H, 1]
            # layout costs 64 four-byte descriptors and stalls the queue)
            bq_sb = constp.tile([1, H], BF16, tag="bq")
            nc.scalar.dma_start(bq_sb[:], bq_d[:, :])
            bv_sb = constp.tile([1, H], BF16, tag="bv")
            nc.scalar.dma_start(bv_sb[:], bv_d[:, :])

            xh = [xinp.tile([P, SH_COLS], BF16, tag="xh", name=f"xh_{j}")
                  for j in range(NCORES)]

            def xsrc(j, lo, hi):
                return xt_d[:, SH_COLS * j + lo:SH_COLS * j + hi]

            HALF = SH_COLS // 2
            QTR = SH_COLS // 4
            # shard 0 split sync/gpsimd (the scalar DMA queue moves bulk
            # data ~3-4x slower than sync/gpsimd — measured — so it only
            # ever carries the small weights and SBUF-local copies)
            nc.gpsimd.dma_start(xh[0][:, 0:QTR], xsrc(0, 0, QTR))
            nc.sync.dma_start(xh[0][:, QTR:HALF], xsrc(0, QTR, HALF))
            nc.gpsimd.dma_start(xh[0][:, HALF:3 * QTR],
                                xsrc(0, HALF, 3 * QTR))
            nc.sync.dma_start(xh[0][:, 3 * QTR:SH_COLS],
                              xsrc(0, 3 * QTR, SH_COLS))

            def x_load(j):
                # halves on sync+gpsimd, shard-major order
                nc.sync.dma_start(xh[j][:, 0:HALF], xsrc(j, 0, HALF))
                nc.gpsimd.dma_start(xh[j][:, HALF:SH_COLS],
                                    xsrc(j, HALF, SH_COLS))

            for j in (1, 2, 3):
                x_load(j)
            # shards 4-7 recycle bufs 0-3; their triggers wait on buffer
            # release, so they are EMITTED LAZILY (in the shard loop
            # below) to avoid parking a blocked trigger at a queue head.

            # ---- constants ----------------------------------------------
            id_bf = constp.tile([P, P], BF16, tag="id_bf")
            masks.make_identity(nc, id_bf[:])
            ones1 = constp.tile([1, 512], BF16, tag="ones1")
            nc.vector.memset(ones1[:], 1.0)

            # kvT for all shards: [p, shard, key-block, col]
            #   even blocks (m-half 0): k on partitions 0:64, v on 64:128
            #   odd  blocks (m-half 1): v on partitions 0:64, k on 64:128
            kvT_all = qkvp.tile([P, NCORES, MT, P], BF16, tag="kvT")
            # v natural [key, h|1] tiles; ones col feeds the row-sums
            if fp8v:
                v_all = qkvp.tile([P, NCORES * NPAIR, 2, H + 1], F8,
                                  tag="v_nat")
                nc.vector.memset(v_all[:, :, :, H:H + 1], 1.0)
            else:
                v_all = qkvp.tile([P, NCH, H + 1], BF16, tag="v_nat")
                nc.vector.memset(v_all[:, :, H:H + 1], 1.0)
            # qTa = [q; 0] (for even key blocks), qTb = [0; q] (odd)
            qTa = qkvp.tile([P, ML], BF16, tag="qTa")
            qTb = qkvp.tile([P, ML], BF16, tag="qTb")
            if not rowtile:
                nc.vector.memset(qTa[H:P, :], 0.0)
                nc.vector.memset(qTb[0:H, :], 0.0)

            with (
                tc.tile_pool(name="ps_a", bufs=2, space="PSUM") as ps_a,
                tc.tile_pool(name="ps_sT", bufs=2, space="PSUM") as ps_sT,
                tc.tile_pool(name="ps_oT", bufs=1, space="PSUM") as ps_oT,
            ):
                # ---- shard 0: kv + q interleaved chunk-by-chunk --------
                kv_accs = [ps_a.tile([P, 512], F32, tag="acc",
                                     name=f"acc_kv0_{h2}")
                           for h2 in range(2)]
                q_accs = [ps_sT.tile([H, 512], F32, tag="sT",
                                     name=f"acc_q_{h2}")
                          for h2 in range(2)]
                for ch in range(CCH):
                    for h2 in range(2):
                        nc.tensor.matmul(
                            kv_accs[h2][:],
                            wkv_sb[:, h2, P * ch:P * (ch + 1)],
                            xh[0][:, ML * ch + 512 * h2:
                                  ML * ch + 512 * (h2 + 1)],
                            start=(ch == 0), stop=(ch == CCH - 1))
                    for h2 in range(2):
                        nc.tensor.matmul(
                            q_accs[h2][:], wq_sb[:, H * ch:H * (ch + 1)],
                            xh[0][:, ML * ch + 512 * h2:
                                  ML * ch + 512 * (h2 + 1)],
                            start=(ch == 0), stop=(ch == CCH - 1))

                def kv_finish(j, kv_accs):
                    # PSUM -> SBUF kvT copies, then v natural tiles
                    for h2 in range(2):
                        eng = nc.scalar if (j + h2) % 2 == 0 else nc.vector
                        (eng.copy if eng is nc.scalar
                         else eng.tensor_copy)(
                            kvT_all[:, j, 4 * h2:4 * (h2 + 1), :],
                            kv_accs[h2][:])
                    # v natural via PE transposes (XBAR SBUF->SBUF DMA
                    # tried instead: its 256B-descriptor transfers choke
                    # the shared DMA engines and starve the x stream)
                    for b in range(MT):
                        # even blocks: v at 64:128; odd: v at 0:64
                        rows = slice(H, P) if b < 4 else slice(0, H)
                        vps = ps_a.tile([P, H], BF16, tag="acc",
                                        name=f"vps_{j}_{b}")
                        nc.tensor.transpose(
                            vps[:], kvT_all[rows, j, b, :],
                            id_bf[rows, rows])
                        eng = nc.scalar if b % 2 == 0 else nc.vector
                        if fp8v:
                            pr, i = (b, 0) if b < 4 else (b - 4, 1)
                            dst = v_all[:, NPAIR * j + pr, i, 0:H]
                        else:
                            dst = v_all[:, MT * j + b, 0:H]
                        (eng.copy if eng is nc.scalar
                         else eng.tensor_copy)(dst, vps[:])

                kv_finish(0, kv_accs)
                # bq enters as a rank-1 accumulation bq (x) ones into the
                # q PSUM chain (avoids a [H,1]-layout bias DMA)
                for h2 in range(2):
                    nc.tensor.matmul(q_accs[h2][:], bq_sb[:], ones1[:],
                                     start=False, stop=True,
                                     skip_group_check=True)
                for h2 in range(2):
                    nc.vector.tensor_copy(
                        qTa[0:H, 512 * h2:512 * (h2 + 1)], q_accs[h2][:])
                # duplicate q rows into qTb's bottom half (SBUF->SBUF
                # DMA on scalar: its queue is empty by now, so the dup
                # lands right after qTa is written)
                nc.scalar.dma_start(qTb[H:P, :], qTa[0:H, :])

                # ---- attention ------------------------------------------
                oT = ps_oT.tile([H + 1, ML], F32, tag="oT")
                epairs = []   # (eT tile-ish, global pair idx)

                def accum_v(ep, gp):
                    if fp8v:
                        eTb, = ep
                        for h2 in range(2):
                            msl = slice(512 * h2, 512 * (h2 + 1))
                            nc.tensor.matmul(
                                oT[:, msl], v_all[:, gp, :, :],
                                eTb[:, :, msl],
                                start=(gp == 0),
                                stop=(gp == NCORES * NPAIR - 1),
                                perf_mode=DR, skip_group_check=True)
                    else:
                        eA, eB, gbA, gbB = ep
                        for eT, gb in ((eA, gbA), (eB, gbB)):
                            for h2 in range(2):
                                msl = slice(512 * h2, 512 * (h2 + 1))
                                nc.tensor.matmul(
                                    oT[:, msl], v_all[:, gb, :],
                                    eT[:, msl],
                                    start=(gb == 0), stop=(gb == NCH - 1),
                                    skip_group_check=True)

                def attn_pair(j, pr):
                    gp = NPAIR * j + pr
                    last = gp == NCORES * NPAIR - 1
                    sA = ps_sT.tile([P, ML], F32, tag="sT",
                                    name=f"sA_{gp}")
                    sB = ps_sT.tile([P, ML], F32, tag="sT",
                                    name=f"sB_{gp}")
                    for h2 in range(2):
                        msl = slice(512 * h2, 512 * (h2 + 1))
                        if rowtile:
                            nc.tensor.matmul(
                                sA[:, msl], kvT_all[0:H, j, pr, :],
                                qTa[0:H, msl], start=True, stop=True,
                                tile_position=(0, 0))
                            nc.tensor.matmul(
                                sB[:, msl], kvT_all[H:P, j, 4 + pr, :],
                                qTb[H:P, msl], start=True, stop=True,
                                tile_position=(64, 0))
                        else:
                            nc.tensor.matmul(
                                sA[:, msl], kvT_all[:, j, pr, :],
                                qTa[:, msl], start=True, stop=True)
                            nc.tensor.matmul(
                                sB[:, msl], kvT_all[:, j, 4 + pr, :],
                                qTb[:, msl], start=True, stop=True)
                    # exp: block A on ScalarE (native), block B on
                    # VectorE (Schraudolph).  Last pair: split each
                    # block's halves across both engines (critical path).
                    if fp8v:
                        eTi = eTp.tile([P, 2, ML], U8, tag="eT",
                                       name=f"eT_{gp}")
                        eTb = eTi.bitcast(F8)
                        if last:
                            nc.scalar.activation(eTb[:, 0, 0:512],
                                                 sA[:, 0:512], AF.Exp,
                                                 scale=SCALE)
                            nc.vector.tensor_scalar(
                                eTi[:, 0, 512:ML], sA[:, 512:ML],
                                A8, B8, op0=MULT, op1=ADD)
                            nc.scalar.activation(eTb[:, 1, 0:512],
                                                 sB[:, 0:512], AF.Exp,
                                                 scale=SCALE)
                            nc.vector.tensor_scalar(
                                eTi[:, 1, 512:ML], sB[:, 512:ML],
                                A8, B8, op0=MULT, op1=ADD)
                        else:
                            nc.scalar.activation(eTb[:, 0, :], sA[:],
                                                 AF.Exp, scale=SCALE)
                            nc.vector.tensor_scalar(
                                eTi[:, 1, :], sB[:], A8, B8,
                                op0=MULT, op1=ADD)
                        epairs.append(((eTb,), gp))
                    else:
                        eA = eTp.tile([P, ML], BF16, tag="eT",
                                      name=f"eA_{gp}")
                        eBi = eTp.tile([P, ML], I16, tag="eT",
                                       name=f"eB_{gp}")
                        eB = eBi.bitcast(BF16)
                        if last:
                            nc.scalar.activation(eA[:, 0:512],
                                                 sA[:, 0:512], AF.Exp,
                                                 scale=SCALE)
                            nc.vector.tensor_scalar(
                                eA.bitcast(I16)[:, 512:ML], sA[:, 512:ML],
                                A16, B16, op0=MULT, op1=ADD)
                            nc.scalar.activation(eB[:, 0:512],
                                                 sB[:, 0:512], AF.Exp,
                                                 scale=SCALE)
                            nc.vector.tensor_scalar(
                                eBi[:, 512:ML], sB[:, 512:ML],
                                A16, B16, op0=MULT, op1=ADD)
                        else:
                            nc.scalar.activation(eA[:], sA[:], AF.Exp,
                                                 scale=SCALE)
                            nc.vector.tensor_scalar(eBi[:], sB[:],
                                                    A16, B16,
                                                    op0=MULT, op1=ADD)
                        epairs.append(
                            ((eA, eB, MT * j + pr, MT * j + 4 + pr), gp))
                    if len(epairs) > PIPE_P:
                        accum_v(*epairs[-1 - PIPE_P])

                def kv_chunk(j, ch, kv_accs):
                    for h2 in range(2):
                        nc.tensor.matmul(
                            kv_accs[h2][:],
                            wkv_sb[:, h2, P * ch:P * (ch + 1)],
                            xh[j][:, ML * ch + 512 * h2:
                                  ML * ch + 512 * (h2 + 1)],
                            start=(ch == 0), stop=(ch == CCH - 1))

                # per shard: 4 attention pairs interleaved with the next
                # shard's kv chunks, then its copies/transposes.  The
                # recycled-buffer DMAs (shards 4-7) are emitted as the
                # buffer they reuse is released (shard j-4's xh frees
                # after its kv+q reads complete).
                for j in range(NCORES):
                    nxt = None
                    if j + 1 < NCORES:
                        nxt = [ps_a.tile([P, 512], F32, tag="acc",
                                         name=f"acc_kv{j + 1}_{h2}")
                               for h2 in range(2)]
                    for pr in range(NPAIR):
                        attn_pair(j, pr)
                        if nxt is not None:
                            kv_chunk(j + 1, 2 * pr, nxt)
                            kv_chunk(j + 1, 2 * pr + 1, nxt)
                    if nxt is not None:
                        kv_finish(j + 1, nxt)
                    # recycled-buffer x DMAs, emitted after kv_finish so
                    # the v-transpose DMAs stay ahead of them in-queue
                    if j + 4 < NCORES:
                        x_load(j + 4)
                for ep in epairs[-PIPE_P:]:
                    accum_v(*ep)

                # ---- finalize (stay transposed) -------------------------
                # den row -> SBUF (bf16), numerator += bv (x) den,
                # den broadcast via ones (x) den, reciprocal, multiply.
                den_sb = finp.tile([1, ML], BF16, tag="den")
                nc.vector.tensor_copy(den_sb[:], oT[H:H + 1, :])
                for h2 in range(2):
                    msl = slice(512 * h2, 512 * (h2 + 1))
                    nc.tensor.matmul(oT[0:H, msl], bv_sb[:],
                                     den_sb[:, msl], start=False,
                                     stop=True, skip_group_check=True)
                denb_ps = ps_sT.tile([P, ML], F32, tag="sT", name="denb")
                for h2 in range(2):
                    msl = slice(512 * h2, 512 * (h2 + 1))
                    nc.tensor.matmul(denb_ps[:, msl], ones1[:, 0:P],
                                     den_sb[:, msl], start=True,
                                     stop=True)
                rcpb = finp.tile([P, ML], F32, tag="rcpb")
                nc.vector.reciprocal_approx_fast(rcpb[:], denb_ps[:])
                out_f = finp.tile([H, ML], F32, tag="out_f")
                nc.vector.tensor_tensor(out_f[:], oT[0:H, :],
                                        rcpb[0:H, :], op=MULT)
                nc.sync.dma_start(out_d[:, 0:512], out_f[:, 0:512])
                nc.scalar.dma_start(out_d[:, 512:ML], out_f[:, 512:ML])

    nc.compile()
    return nc


def _get_nc():
    key = (ROWTILE, FP8V, FP8_U8_TS)
    if key not in _CACHE:
        _CACHE[key] = _build(*key)
    return _CACHE[key]


# query/key 128-row block interleave within each shard: [0,2,4,6,1,3,5,7]
_PERM = np.concatenate([np.arange(0, MT, 2), np.arange(1, MT, 2)])


def _prep_inputs(inputs):
    import ml_dtypes

    bf16 = ml_dtypes.bfloat16
    wk = np.asarray(inputs["Wk"], dtype=np.float32)
    wv = np.asarray(inputs["Wv"], dtype=np.float32)
    wkvA = np.concatenate([wk, wv], axis=1).astype(bf16)
    wkvB = np.concatenate([wv, wk], axis=1).astype(bf16)
    # chunk-major packing [P, CCH*P] then stack A/B
    packA = np.ascontiguousarray(
        wkvA.reshape(CCH, P, P).transpose(1, 0, 2).reshape(P, CCH * P))
    packB = np.ascontiguousarray(
        wkvB.reshape(CCH, P, P).transpose(1, 0, 2).reshape(P, CCH * P))
    wkv_p = np.ascontiguousarray(
        np.stack([packA, packB], axis=1))            # [P, 2, CCH*P]
    wq = np.asarray(inputs["Wq"], dtype=np.float32).astype(bf16)
    wq_p = np.ascontiguousarray(
        wq.reshape(CCH, P, H).transpose(1, 0, 2).reshape(P, CCH * H))
    bq = np.ascontiguousarray(
        np.asarray(inputs["bq"], dtype=np.float32).astype(bf16)
    ).reshape(1, H)
    bv = np.ascontiguousarray(
        np.asarray(inputs["bv"], dtype=np.float32).astype(bf16)
    ).reshape(1, H)

    x = np.asarray(inputs["x"], dtype=np.float32)
    SH_COLS = CCH * ML
    shards = []
    for j in range(NCORES):
        xs = x[ML * j:ML * (j + 1)].astype(bf16)      # [m, d]
        # interleave the 128-row blocks: even blocks first, then odd
        xs = xs.reshape(MT, P, D)[_PERM].reshape(ML, D)
        t = xs.T.reshape(CCH, P, ML)                  # [c, p, m]
        shards.append(np.ascontiguousarray(
            t.transpose(1, 0, 2).reshape(P, SH_COLS)))
    in_maps = []
    for i in range(NCORES):
        # rotate so shard slot 0 is core i's own rows
        xt = np.concatenate([shards[(i + j) % NCORES]
                             for j in range(NCORES)], axis=1)
        in_maps.append({
            "xt": np.ascontiguousarray(xt), "wkv": wkv_p, "wq": wq_p,
            "bq": bq, "bv": bv,
        })
    return in_maps


def _run(inputs, trace=False, **kw):
    from concourse.bass_utils import run_bass_kernel_spmd

    nc = _get_nc()
    in_maps = _prep_inputs(inputs)
    res = run_bass_kernel_spmd(nc, in_maps, core_ids=list(range(NCORES)),
                               trace=trace, **kw)
    inv = np.argsort(_PERM)
    parts = []
    for i in range(NCORES):
        o = res.results[i]["out"]                     # [H, ML]
        o = np.ascontiguousarray(o.T)                 # [ML, H] permuted
        parts.append(o.reshape(MT, P, H)[inv].reshape(ML, H))
    return np.concatenate(parts, axis=0), res


def kernel(x, Wq, bq, Wk, bk, Wv, bv):
    out, _ = _run({"x": x, "Wq": Wq, "bq": bq, "Wk": Wk, "Wv": Wv, "bv": bv})
    return out
